# revision 12
# baseline (speedup 1.0000x reference)
"""Trainium2 Bass kernel for nn_CrossAttention_31078383354530.

Reference computation (b=2, n=m=2048, qd=1024, cd=768, heads=8, dh=128):
    q = x @ Wq; k = ctx @ Wk; v = ctx @ Wv  (split into 8 heads of 128)
    sim = (q @ k^T) * dh**-0.5 over the FLATTENED (b*n)=4096 token axis
    attn = softmax((sim - mean)*1.5 + mean) == softmax(1.5*scale*(q@k^T))
        exactly (the mean-centering is a per-row constant shift)
    out = attn @ v -> merge heads -> y = out @ Wout + bout

Sharding (v4, head-ownership via AllToAll):
  Core c owns head c (the per-core Wq column slice is supplied as input
  data, keeping the program SPMD).  Each core projects K/V for its own
  512 context tokens (all heads) and AllToAll-redistributes them by head:
  wire cost ~2.6MB/core vs ~14MB/core for the head-replicated AllGather
  scheme (each K/V byte has exactly one consumer).  Each core then runs
  its head's attention for ALL 4096 queries (q = full x @ Wqh, x supplied
  in full), producing out_h[d, 4096].  A second small AllToAll (1MB)
  redistributes outputs back to token-sharding, and each core applies the
  full Wout to its own 512-token slice -> disjoint row-slices, no host
  reduction.

Schedule notes:
  * A tiny dummy AllGather at t=0 absorbs the first-collective barrier.
  * Attention inner loop per 512-query chunk: 3-j-tile PSUM groups
    (sim -> Scalar exp -> PV), softmax row-sums accumulated on the DVE
    with 1024-wide bf16 adds into a two-half accumulator, partition-
    reduced by a single tiny ones-matmul; reciprocal batched per chunk
    pair; normalization runs on SBUF copies off the critical path.
"""

import sys

if "/opt/trn_rl_repo" not in sys.path:
    sys.path.insert(0, "/opt/trn_rl_repo")

import ml_dtypes
import numpy as np

import concourse.bass as bass  # noqa: F401
import concourse.mybir as mybir
import concourse.tile as tile
from concourse import bacc, bass_utils

F32 = mybir.dt.float32
BF16 = mybir.dt.bfloat16
AF = mybir.ActivationFunctionType

P = 128
N_CORES = 8
HEADS = 8
DH = 128
TOK = 4096             # b*n flattened token axis (attention mixes batches!)
SLICE = TOK // N_CORES  # 512 tokens per core
QD = 1024
CD = 768
INNER = 1024
KC = QD // P           # 8 qd chunks
CC = CD // P           # 6 cd chunks
JT = TOK // P          # 32 j-tiles
GRP = 3                # j-tiles per exp group ([128, 1536] psum, 3 banks)
NQC = TOK // SLICE     # 8 query chunks (= old "heads" loop)
TAU_SCALE = 1.5 * (DH ** -0.5)

_CACHE = {}


def _build():
    nc = bacc.Bacc(num_devices=N_CORES)

    xT = nc.declare_dram_parameter("xT", [QD, TOK], BF16, isOutput=False)
    Wqh = nc.declare_dram_parameter("Wqh", [QD, DH], BF16, isOutput=False)
    cTs = nc.declare_dram_parameter("cTs", [CD, SLICE], BF16, isOutput=False)
    Wk = nc.declare_dram_parameter("Wk", [CD, INNER], BF16, isOutput=False)
    Wv = nc.declare_dram_parameter("Wv", [CD, INNER], BF16, isOutput=False)
    Wout = nc.declare_dram_parameter("Wout", [INNER, QD], BF16, isOutput=False)
    boutT = nc.declare_dram_parameter("boutT", [P, KC], F32, isOutput=False)
    yT = nc.declare_dram_parameter("yT", [KC, P, SLICE], F32, isOutput=True)

    rg = [list(range(N_CORES))]

    with tile.TileContext(nc) as tc:
        with (
            tc.tile_pool(name="const", bufs=1) as const,
            tc.tile_pool(name="sb", bufs=1) as sb,
            tc.tile_pool(name="ps", bufs=1, space="PSUM") as ps,
            tc.tile_pool(name="dram", bufs=1, space="DRAM") as dram,
        ):
            akin = dram.tile([HEADS, P, SLICE], BF16, name="akin")
            akout = dram.tile([HEADS, P, SLICE], BF16, name="akout")
            avin = dram.tile([HEADS, P, SLICE], BF16, name="avin")
            avout = dram.tile([HEADS, P, SLICE], BF16, name="avout")
            aoin = dram.tile([NQC, P, SLICE], BF16, name="aoin")
            aoout = dram.tile([NQC, P, SLICE], BF16, name="aoout")

            # ---- dummy collective: absorbs the first-collective barrier
            # while the projection phase runs (contents never used)
            dmy_in = dram.tile([P, 8], BF16, name="dmy_in")
            dmy_out = dram.tile([N_CORES, P, 8], BF16, addr_space="Shared",
                                name="dmy_out")
            nc.gpsimd.collective_compute(
                "AllGather", mybir.AluOpType.bypass, replica_groups=rg,
                ins=[dmy_in.opt()], outs=[dmy_out.opt()])

            ones2 = const.tile([P, 4], BF16, name="ones2")
            nc.vector.memset(ones2[:], 1.0)
            nc.vector.memset(ones2[:, 1:3], 0.0)
            bout_sb = const.tile([P, KC], F32, name="bout_sb")
            nc.sync.dma_start(bout_sb[:], boutT[:, :])

            # ---- K/V projection inputs ----
            cts = []
            for k in range(CC):
                t = sb.tile([P, SLICE], BF16, name=f"cts{k}", tag="cts", bufs=CC)
                nc.sync.dma_start(t[:], cTs[k * P:(k + 1) * P, :])
                cts.append(t)
            wkt = []
            for k in range(CC):
                t = sb.tile([P, INNER], BF16, name=f"wkt{k}", tag="wkt", bufs=CC)
                nc.sync.dma_start(t[:], Wk[k * P:(k + 1) * P, :])
                wkt.append(t)
            wvt = []
            for k in range(CC):
                t = sb.tile([P, INNER], BF16, name=f"wvt{k}", tag="wvt", bufs=CC)
                nc.sync.dma_start(t[:], Wv[k * P:(k + 1) * P, :])
                wvt.append(t)

            # ---- K projection (all heads, own ctx slice) -> A2A by head ----
            for h in range(HEADS):
                kps = ps.tile([P, GRP * SLICE], F32, name=f"kps{h}",
                              tag="sim", bufs=2)
                for k in range(CC):
                    nc.tensor.matmul(kps[:, :SLICE],
                                     wkt[k][:, h * DH:(h + 1) * DH],
                                     cts[k][:],
                                     start=(k == 0), stop=(k == CC - 1))
                ksb = sb.tile([P, SLICE], BF16, name=f"ksb{h}", tag="ksb",
                              bufs=4)
                nc.vector.tensor_copy(ksb[:], kps[:, :SLICE])
                nc.sync.dma_start(akin[h], ksb[:])
            nc.gpsimd.collective_compute(
                "AllToAll", mybir.AluOpType.bypass, replica_groups=rg,
                ins=[akin.opt()], outs=[akout.opt()])

            # ---- V projection (all heads, own ctx slice) -> A2A by head ----
            for p_ in range(HEADS // 2):
                vsb = sb.tile([P, 4 * 2 * DH], BF16, name=f"vsb{p_}",
                              tag="vsb", bufs=2)
                for tt in range(SLICE // P):
                    vps = ps.tile([P, GRP * SLICE], F32, name=f"vps{p_}_{tt}",
                                  tag="sim", bufs=2)
                    for k in range(CC):
                        nc.tensor.matmul(
                            vps[:, :2 * DH],
                            cts[k][:, tt * P:(tt + 1) * P],
                            wvt[k][:, p_ * 2 * DH:(p_ + 1) * 2 * DH],
                            start=(k == 0), stop=(k == CC - 1))
                    nc.vector.tensor_copy(
                        vsb[:, tt * 2 * DH:(tt + 1) * 2 * DH], vps[:, :2 * DH])
                vsb3 = vsb[:].rearrange("p (t c) -> p t c", t=4)
                for hh in range(2):
                    nc.sync.dma_start(avin[2 * p_ + hh],
                                      vsb3[:, :, hh * DH:(hh + 1) * DH])
            nc.gpsimd.collective_compute(
                "AllToAll", mybir.AluOpType.bypass, replica_groups=rg,
                ins=[avin.opt()], outs=[avout.opt()])

            # ---- Q projection: own head, ALL 4096 queries ----
            xts = []
            for k in range(KC):
                t = sb.tile([P, TOK], BF16, name=f"xts{k}", tag="xts", bufs=KC)
                nc.sync.dma_start(t[:], xT[k * P:(k + 1) * P, :])
                xts.append(t)
            wqh = []
            for k in range(KC):
                t = sb.tile([P, DH], BF16, name=f"wqh{k}", tag="wqh", bufs=KC)
                nc.sync.dma_start(t[:], Wqh[k * P:(k + 1) * P, :])
                wqh.append(t)
            # Wout chunks early (nothing depends on them until the tail)
            wo = []
            for cc in range(KC):
                t = sb.tile([P, KC, DH], BF16, name=f"wo{cc}", tag="wo", bufs=KC)
                nc.sync.dma_start(
                    t[:],
                    Wout.ap()[:, cc * DH:(cc + 1) * DH].rearrange(
                        "(k p) c -> p k c", p=P))
                wo.append(t)
            qsb = []
            for qc in range(NQC):
                qps = ps.tile([P, GRP * SLICE], F32, name=f"qps{qc}", tag="sim",
                              bufs=2)
                for k in range(KC):
                    nc.tensor.matmul(
                        qps[:, :SLICE], wqh[k][:],
                        xts[k][:, qc * SLICE:(qc + 1) * SLICE],
                        start=(k == 0), stop=(k == KC - 1))
                qt = sb.tile([P, SLICE], BF16, name=f"qsb{qc}", tag="qsb",
                             bufs=NQC)
                nc.vector.tensor_copy(qt[:], qps[:, :SLICE])
                qsb.append(qt)

            # ---- gathered K/V for my head (once) ----
            kh = sb.tile([P, TOK], BF16, name="kh", tag="kh", bufs=1)
            nc.sync.dma_start(
                kh[:].rearrange("p (r s) -> p r s", r=N_CORES),
                akout[:].rearrange("r p s -> p r s"))
            vh = sb.tile([P, TOK], BF16, name="vh", tag="vh", bufs=1)
            nc.sync.dma_start(
                vh[:].rearrange("p (r s) -> p r s", r=N_CORES),
                avout[:].rearrange("r p s -> p r s"))

            # ---- attention, one 512-query chunk at a time ----
            groups = []
            j0 = 0
            while j0 < JT:
                groups.append(list(range(j0, min(j0 + GRP, JT))))
                j0 += GRP

            pvs_pair = [None, None]
            rs2_pair = None
            for qc in range(NQC):
                p_, hh = qc // 2, qc % 2
                pv_ps = ps.tile([P, SLICE], F32, name=f"pv{qc}", tag="pv",
                                bufs=1)
                acc2 = sb.tile([P, 2 * SLICE], BF16, name=f"acc{qc}",
                               tag="acc", bufs=2)
                if hh == 0:
                    rs2_ps = ps.tile([2, SLICE], F32, name=f"rs{p_}", tag="rs",
                                     bufs=1)
                    rs2_pair = rs2_ps
                else:
                    rs2_ps = rs2_pair
                for g, js in enumerate(groups):
                    sim_ps = ps.tile([P, GRP * SLICE], F32, name=f"sim{qc}_{g}",
                                     tag="sim", bufs=2)
                    for jj, j in enumerate(js):
                        nc.tensor.matmul(
                            sim_ps[:, jj * SLICE:(jj + 1) * SLICE],
                            kh[:, j * P:(j + 1) * P], qsb[qc][:],
                            start=True, stop=True)
                    at = sb.tile([P, GRP * SLICE], BF16, name=f"at{qc}_{g}",
                                 tag="at", bufs=4)
                    nc.scalar.activation(at[:, :len(js) * SLICE],
                                         sim_ps[:, :len(js) * SLICE], AF.Exp,
                                         scale=TAU_SCALE)
                    for jj, j in enumerate(js):
                        nc.tensor.matmul(pv_ps[:], vh[:, j * P:(j + 1) * P],
                                         at[:, jj * SLICE:(jj + 1) * SLICE],
                                         start=(j == 0), stop=(j == JT - 1))
                    # DVE row-sum accumulation (1024-wide, 2x mode)
                    w = min(len(js), 2) * SLICE
                    if g == 0:
                        nc.vector.tensor_copy(acc2[:, :w], at[:, :w])
                    else:
                        nc.vector.tensor_tensor(acc2[:, :w], acc2[:, :w],
                                                at[:, :w],
                                                mybir.AluOpType.add)
                    if len(js) == 3:
                        nc.vector.tensor_tensor(
                            acc2[:, :SLICE], acc2[:, :SLICE],
                            at[:, 2 * SLICE:3 * SLICE], mybir.AluOpType.add)
                # partition-reduce on the PE into the pair's [2,512] rowsum
                st = ones2[:, 2 * hh:2 * hh + 2]
                nc.tensor.matmul(rs2_ps[:], st, acc2[:, :SLICE],
                                 start=(hh == 0), stop=False)
                nc.tensor.matmul(rs2_ps[:], st, acc2[:, SLICE:2 * SLICE],
                                 start=False, stop=(hh == 1))
                # drain pv quickly; normalize per chunk-PAIR off critical path
                pvs = sb.tile([P, SLICE], F32, name=f"pvs{qc}", tag="pvs",
                              bufs=2)
                nc.vector.tensor_copy(pvs[:], pv_ps[:])
                pvs_pair[hh] = pvs
                if hh == 1:
                    recip2 = sb.tile([2, SLICE], F32, name=f"recip{p_}",
                                     tag="recip", bufs=2)
                    nc.vector.reciprocal(recip2[:], rs2_ps[:])
                    recip_b = sb.tile([1, SLICE], F32, name=f"recipb{p_}",
                                      tag="recipb", bufs=2)
                    nc.sync.dma_start(recip_b[:], recip2[1:2])
                    for e in range(2):
                        bc = sb.tile([P, SLICE], F32, name=f"bc{qc}_{e}",
                                     tag="bc", bufs=2)
                        nc.gpsimd.partition_broadcast(
                            bc[:], recip2[0:1] if e == 0 else recip_b[:])
                        ot = sb.tile([P, SLICE], BF16, name=f"osb{qc}_{e}",
                                     tag="osb", bufs=2)
                        nc.vector.tensor_tensor(ot[:], pvs_pair[e][:], bc[:],
                                                mybir.AluOpType.mult)
                        nc.sync.dma_start(aoin[2 * p_ + e], ot[:])

            # ---- redistribute outputs back to token-sharding ----
            nc.gpsimd.collective_compute(
                "AllToAll", mybir.AluOpType.bypass, replica_groups=rg,
                ins=[aoin.opt()], outs=[aoout.opt()])
            o2 = []
            for ic in range(HEADS):
                t = sb.tile([P, SLICE], BF16, name=f"o2_{ic}", tag="o2",
                            bufs=HEADS)
                nc.sync.dma_start(t[:], aoout[ic])
                o2.append(t)

            # ---- final projection: yT[cc] = Wout[:, cc]^T @ out^T + bout ----
            for cc in range(KC):
                yps = ps.tile([P, SLICE], F32, name=f"yps{cc}",
                              tag=("pv" if cc % 2 == 0 else "rs"), bufs=1)
                for ic in range(HEADS):
                    nc.tensor.matmul(yps[:], wo[cc][:, ic], o2[ic][:],
                                     start=(ic == 0), stop=(ic == HEADS - 1))
                yt = sb.tile([P, SLICE], F32, name=f"yt{cc}", tag="yt", bufs=2)
                nc.scalar.activation(yt[:], yps[:], AF.Identity,
                                     bias=bout_sb[:, cc:cc + 1], scale=1.0)
                nc.sync.dma_start(yT.ap()[cc], yt[:])

    nc.compile()
    return nc


def _get_nc():
    if "nc" not in _CACHE:
        _CACHE["nc"] = _build()
    return _CACHE["nc"]


def _bf16(a):
    return np.ascontiguousarray(np.asarray(a, np.float32).astype(ml_dtypes.bfloat16))


def _prep_in_maps(x, context, Wq, Wk, Wv, Wout, bout):
    x_f = np.asarray(x, dtype=np.float32).reshape(TOK, QD)
    c_f = np.asarray(context, dtype=np.float32).reshape(TOK, CD)
    xT = _bf16(x_f.T)
    Wq = _bf16(Wq)
    Wk = _bf16(Wk)
    Wv = _bf16(Wv)
    Wout = _bf16(Wout)
    boutT = np.ascontiguousarray(
        np.asarray(bout, dtype=np.float32).reshape(KC, P).T)
    in_maps = []
    for c in range(N_CORES):
        sl = slice(c * SLICE, (c + 1) * SLICE)
        in_maps.append({
            "xT": xT,
            "Wqh": np.ascontiguousarray(Wq[:, c * DH:(c + 1) * DH]),
            "cTs": _bf16(c_f[sl].T),
            "Wk": Wk, "Wv": Wv, "Wout": Wout, "boutT": boutT,
        })
    return in_maps


def _assemble(results):
    y = np.empty((TOK, QD), dtype=np.float32)
    for c in range(N_CORES):
        yt = results[c]["yT"]   # [KC, P, SLICE]
        y[c * SLICE:(c + 1) * SLICE] = (
            yt.transpose(2, 0, 1).reshape(SLICE, QD))
    return y.reshape(2, TOK // 2, QD)


def run(inputs, trace=False, **kw):
    nc = _get_nc()
    in_maps = _prep_in_maps(**inputs)
    res = bass_utils.run_bass_kernel_spmd(
        nc, in_maps, core_ids=list(range(N_CORES)), trace=trace, **kw)
    return _assemble(res.results), res


def kernel(**inputs):
    out, _ = run(inputs, trace=False)
    return out


# revision 14
# speedup vs baseline: 1.0152x; 1.0152x over previous
"""Trainium2 Bass kernel for nn_CrossAttention_31078383354530.

Reference computation (b=2, n=m=2048, qd=1024, cd=768, heads=8, dh=128):
    q = x @ Wq; k = ctx @ Wk; v = ctx @ Wv  (split into 8 heads of 128)
    sim = (q @ k^T) * dh**-0.5 over the FLATTENED (b*n)=4096 token axis
    attn = softmax((sim - mean)*1.5 + mean) == softmax(1.5*scale*(q@k^T))
        exactly (the mean-centering is a per-row constant shift)
    out = attn @ v -> merge heads -> y = out @ Wout + bout

Sharding (v4, head-ownership via AllToAll):
  Core c owns head c (the per-core Wq column slice is supplied as input
  data, keeping the program SPMD).  Each core projects K/V for its own
  512 context tokens (all heads) and AllToAll-redistributes them by head:
  wire cost ~2.6MB/core vs ~14MB/core for the head-replicated AllGather
  scheme (each K/V byte has exactly one consumer).  Each core then runs
  its head's attention for ALL 4096 queries (q = full x @ Wqh, x supplied
  in full), producing out_h[d, 4096].  A second small AllToAll (1MB)
  redistributes outputs back to token-sharding, and each core applies the
  full Wout to its own 512-token slice -> disjoint row-slices, no host
  reduction.

Schedule notes:
  * A tiny dummy AllGather at t=0 absorbs the first-collective barrier.
  * Attention inner loop per 512-query chunk: 3-j-tile PSUM groups
    (sim -> Scalar exp -> PV), softmax row-sums accumulated on the DVE
    with 1024-wide bf16 adds into a two-half accumulator, partition-
    reduced by a single tiny ones-matmul; reciprocal batched per chunk
    pair; normalization runs on SBUF copies off the critical path.
"""

import sys

if "/opt/trn_rl_repo" not in sys.path:
    sys.path.insert(0, "/opt/trn_rl_repo")

import ml_dtypes
import numpy as np

import concourse.bass as bass  # noqa: F401
import concourse.mybir as mybir
import concourse.tile as tile
from concourse import bacc, bass_utils

F32 = mybir.dt.float32
BF16 = mybir.dt.bfloat16
AF = mybir.ActivationFunctionType

P = 128
N_CORES = 8
HEADS = 8
DH = 128
TOK = 4096             # b*n flattened token axis (attention mixes batches!)
SLICE = TOK // N_CORES  # 512 tokens per core
QD = 1024
CD = 768
INNER = 1024
KC = QD // P           # 8 qd chunks
CC = CD // P           # 6 cd chunks
JT = TOK // P          # 32 j-tiles
GRP = 3                # j-tiles per exp group ([128, 1536] psum, 3 banks)
NQC = TOK // SLICE     # 8 query chunks (= old "heads" loop)
TAU_SCALE = 1.5 * (DH ** -0.5)

_CACHE = {}


def _build():
    nc = bacc.Bacc(num_devices=N_CORES)

    xT = nc.declare_dram_parameter("xT", [QD, TOK], BF16, isOutput=False)
    Wqh = nc.declare_dram_parameter("Wqh", [QD, DH], BF16, isOutput=False)
    cTs = nc.declare_dram_parameter("cTs", [CD, SLICE], BF16, isOutput=False)
    Wk = nc.declare_dram_parameter("Wk", [CD, INNER], BF16, isOutput=False)
    Wv = nc.declare_dram_parameter("Wv", [CD, INNER], BF16, isOutput=False)
    Wout = nc.declare_dram_parameter("Wout", [INNER, QD], BF16, isOutput=False)
    boutT = nc.declare_dram_parameter("boutT", [P, KC], F32, isOutput=False)
    yT = nc.declare_dram_parameter("yT", [KC, P, SLICE], F32, isOutput=True)

    rg = [list(range(N_CORES))]

    with tile.TileContext(nc) as tc:
        with (
            tc.tile_pool(name="const", bufs=1) as const,
            tc.tile_pool(name="sb", bufs=1) as sb,
            tc.tile_pool(name="ps", bufs=1, space="PSUM") as ps,
            tc.tile_pool(name="dram", bufs=1, space="DRAM") as dram,
        ):
            akin = dram.tile([HEADS, P, SLICE], BF16, name="akin")
            akout = dram.tile([HEADS, P, SLICE], BF16, name="akout")
            avin = dram.tile([HEADS, P, SLICE], BF16, name="avin")
            avout = dram.tile([HEADS, P, SLICE], BF16, name="avout")
            aoin = dram.tile([NQC, P, SLICE], BF16, name="aoin")
            aoout = dram.tile([NQC, P, SLICE], BF16, name="aoout")

            ones_b = const.tile([P, 1], BF16, name="ones_b")
            nc.vector.memset(ones_b[:], 1.0)
            bout_sb = const.tile([P, KC], F32, name="bout_sb")
            nc.sync.dma_start(bout_sb[:], boutT[:, :])

            # ---- K/V projection inputs ----
            cts = []
            for k in range(CC):
                t = sb.tile([P, SLICE], BF16, name=f"cts{k}", tag="cts", bufs=CC)
                nc.sync.dma_start(t[:], cTs[k * P:(k + 1) * P, :])
                cts.append(t)
            wkt = []
            for k in range(CC):
                t = sb.tile([P, INNER], BF16, name=f"wkt{k}", tag="wkt", bufs=CC)
                nc.sync.dma_start(t[:], Wk[k * P:(k + 1) * P, :])
                wkt.append(t)
            wvt = []
            for k in range(CC):
                t = sb.tile([P, INNER], BF16, name=f"wvt{k}", tag="wvt", bufs=CC)
                nc.sync.dma_start(t[:], Wv[k * P:(k + 1) * P, :])
                wvt.append(t)

            xts = []
            for k in range(KC):
                t = sb.tile([P, TOK], BF16, name=f"xts{k}", tag="xts", bufs=KC)
                nc.sync.dma_start(t[:], xT[k * P:(k + 1) * P, :])
                xts.append(t)
            wqh = []
            for k in range(KC):
                t = sb.tile([P, DH], BF16, name=f"wqh{k}", tag="wqh", bufs=KC)
                nc.sync.dma_start(t[:], Wqh[k * P:(k + 1) * P, :])
                wqh.append(t)
            # Wout chunks early (nothing depends on them until the tail)
            wo = []
            for cc in range(KC):
                t = sb.tile([P, KC, DH], BF16, name=f"wo{cc}", tag="wo", bufs=KC)
                nc.sync.dma_start(
                    t[:],
                    Wout.ap()[:, cc * DH:(cc + 1) * DH].rearrange(
                        "(k p) c -> p k c", p=P))
                wo.append(t)
            # ---- K projection (all heads, own ctx slice) -> A2A by head ----
            for h in range(HEADS):
                kps = ps.tile([P, GRP * SLICE], F32, name=f"kps{h}",
                              tag="sim", bufs=2)
                for k in range(CC):
                    nc.tensor.matmul(kps[:, :SLICE],
                                     wkt[k][:, h * DH:(h + 1) * DH],
                                     cts[k][:],
                                     start=(k == 0), stop=(k == CC - 1))
                ksb = sb.tile([P, SLICE], BF16, name=f"ksb{h}", tag="ksb",
                              bufs=4)
                nc.vector.tensor_copy(ksb[:], kps[:, :SLICE])
                nc.sync.dma_start(akin[h], ksb[:])
            nc.gpsimd.collective_compute(
                "AllToAll", mybir.AluOpType.bypass, replica_groups=rg,
                ins=[akin.opt()], outs=[akout.opt()])

            # ---- V projection (all heads, own ctx slice) -> A2A by head ----
            for p_ in range(HEADS // 2):
                vsb = sb.tile([P, 4 * 2 * DH], BF16, name=f"vsb{p_}",
                              tag="vsb", bufs=2)
                for tt in range(SLICE // P):
                    vps = ps.tile([P, GRP * SLICE], F32, name=f"vps{p_}_{tt}",
                                  tag="sim", bufs=2)
                    for k in range(CC):
                        nc.tensor.matmul(
                            vps[:, :2 * DH],
                            cts[k][:, tt * P:(tt + 1) * P],
                            wvt[k][:, p_ * 2 * DH:(p_ + 1) * 2 * DH],
                            start=(k == 0), stop=(k == CC - 1))
                    nc.vector.tensor_copy(
                        vsb[:, tt * 2 * DH:(tt + 1) * 2 * DH], vps[:, :2 * DH])
                vsb3 = vsb[:].rearrange("p (t c) -> p t c", t=4)
                for hh in range(2):
                    nc.sync.dma_start(avin[2 * p_ + hh],
                                      vsb3[:, :, hh * DH:(hh + 1) * DH])
            nc.gpsimd.collective_compute(
                "AllToAll", mybir.AluOpType.bypass, replica_groups=rg,
                ins=[avin.opt()], outs=[avout.opt()])

            # ---- Q projection: own head, ALL 4096 queries ----
            qsb = []
            for qc in range(NQC):
                qps = ps.tile([P, GRP * SLICE], F32, name=f"qps{qc}", tag="sim",
                              bufs=2)
                for k in range(KC):
                    nc.tensor.matmul(
                        qps[:, :SLICE], wqh[k][:],
                        xts[k][:, qc * SLICE:(qc + 1) * SLICE],
                        start=(k == 0), stop=(k == KC - 1))
                qt = sb.tile([P, SLICE], BF16, name=f"qsb{qc}", tag="qsb",
                             bufs=NQC)
                nc.vector.tensor_copy(qt[:], qps[:, :SLICE])
                qsb.append(qt)

            # ---- gathered K/V for my head (once) ----
            kh = sb.tile([P, TOK], BF16, name="kh", tag="kh", bufs=1)
            nc.sync.dma_start(
                kh[:].rearrange("p (r s) -> p r s", r=N_CORES),
                akout[:].rearrange("r p s -> p r s"))
            vh = sb.tile([P, TOK], BF16, name="vh", tag="vh", bufs=1)
            nc.sync.dma_start(
                vh[:].rearrange("p (r s) -> p r s", r=N_CORES),
                avout[:].rearrange("r p s -> p r s"))

            # ---- attention, one 512-query chunk at a time ----
            groups = []
            j0 = 0
            while j0 < JT:
                groups.append(list(range(j0, min(j0 + GRP, JT))))
                j0 += GRP

            for qc in range(NQC):
                pv_ps = ps.tile([P, SLICE], F32, name=f"pv{qc}", tag="pv",
                                bufs=1)
                acc2 = sb.tile([P, 2 * SLICE], BF16, name=f"acc{qc}",
                               tag="acc", bufs=2)
                for g, js in enumerate(groups):
                    sim_ps = ps.tile([P, GRP * SLICE], F32, name=f"sim{qc}_{g}",
                                     tag="sim", bufs=2)
                    for jj, j in enumerate(js):
                        nc.tensor.matmul(
                            sim_ps[:, jj * SLICE:(jj + 1) * SLICE],
                            kh[:, j * P:(j + 1) * P], qsb[qc][:],
                            start=True, stop=True)
                    at = sb.tile([P, GRP * SLICE], BF16, name=f"at{qc}_{g}",
                                 tag="at", bufs=6)
                    nc.scalar.activation(at[:, :len(js) * SLICE],
                                         sim_ps[:, :len(js) * SLICE], AF.Exp,
                                         scale=TAU_SCALE)
                    for jj, j in enumerate(js):
                        nc.tensor.matmul(pv_ps[:], vh[:, j * P:(j + 1) * P],
                                         at[:, jj * SLICE:(jj + 1) * SLICE],
                                         start=(j == 0), stop=(j == JT - 1))
                    # DVE row-sum accumulation (1024-wide, 2x mode)
                    w = min(len(js), 2) * SLICE
                    if g == 0:
                        nc.vector.tensor_copy(acc2[:, :w], at[:, :w])
                    else:
                        nc.vector.tensor_tensor(acc2[:, :w], acc2[:, :w],
                                                at[:, :w],
                                                mybir.AluOpType.add)
                    if len(js) == 3:
                        nc.vector.tensor_tensor(
                            acc2[:, :SLICE], acc2[:, :SLICE],
                            at[:, 2 * SLICE:3 * SLICE], mybir.AluOpType.add)
                # partition-reduce on the PE into a [1,512] rowsum
                rs_ps = ps.tile([1, SLICE], F32, name=f"rs{qc}", tag="rs",
                                bufs=1)
                nc.tensor.matmul(rs_ps[:], ones_b[:], acc2[:, :SLICE],
                                 start=True, stop=False)
                nc.tensor.matmul(rs_ps[:], ones_b[:], acc2[:, SLICE:2 * SLICE],
                                 start=False, stop=True)
                # drain pv quickly; normalize off the critical path
                pvs = sb.tile([P, SLICE], F32, name=f"pvs{qc}", tag="pvs",
                              bufs=2)
                nc.vector.tensor_copy(pvs[:], pv_ps[:])
                recip = sb.tile([1, SLICE], F32, name=f"recip{qc}",
                                tag="recip", bufs=2)
                nc.vector.reciprocal(recip[:], rs_ps[:])
                bc = sb.tile([P, SLICE], F32, name=f"bc{qc}", tag="bc",
                             bufs=2)
                nc.gpsimd.partition_broadcast(bc[:], recip[:])
                ot = sb.tile([P, SLICE], BF16, name=f"osb{qc}", tag="osb",
                             bufs=2)
                nc.vector.tensor_tensor(ot[:], pvs[:], bc[:],
                                        mybir.AluOpType.mult)
                nc.sync.dma_start(aoin[qc], ot[:])

            # ---- keep the PE warm while A2A-out is in flight: a PE<->DVE
            # ping-pong chain paced by the single-buffer psum bank ----
            for i in range(24):
                wps = ps.tile([P, SLICE], F32, name=f"warm{i}", tag="pv",
                              bufs=1)
                nc.tensor.matmul(wps[0:1], ones_b[:], qsb[0][:],
                                 start=True, stop=True)
                wsb = sb.tile([1, SLICE], F32, name=f"wsb{i}", tag="warm",
                              bufs=2)
                nc.vector.tensor_copy(wsb[:], wps[0:1])

            # ---- redistribute outputs back to token-sharding ----
            nc.gpsimd.collective_compute(
                "AllToAll", mybir.AluOpType.bypass, replica_groups=rg,
                ins=[aoin.opt()], outs=[aoout.opt()])
            o2 = []
            for ic in range(HEADS):
                t = sb.tile([P, SLICE], BF16, name=f"o2_{ic}", tag="o2",
                            bufs=HEADS)
                nc.sync.dma_start(t[:], aoout[ic])
                o2.append(t)

            # ---- final projection: yT[cc] = Wout[:, cc]^T @ out^T + bout ----
            for cc in range(KC):
                yps = ps.tile([P, SLICE], F32, name=f"yps{cc}",
                              tag=("pv" if cc % 2 == 0 else "rs"), bufs=1)
                for ic in range(HEADS):
                    nc.tensor.matmul(yps[:], wo[cc][:, ic], o2[ic][:],
                                     start=(ic == 0), stop=(ic == HEADS - 1))
                yt = sb.tile([P, SLICE], F32, name=f"yt{cc}", tag="yt", bufs=2)
                nc.scalar.activation(yt[:], yps[:], AF.Identity,
                                     bias=bout_sb[:, cc:cc + 1], scale=1.0)
                nc.sync.dma_start(yT.ap()[cc], yt[:])

    nc.compile()
    return nc


def _get_nc():
    if "nc" not in _CACHE:
        _CACHE["nc"] = _build()
    return _CACHE["nc"]


def _bf16(a):
    return np.ascontiguousarray(np.asarray(a, np.float32).astype(ml_dtypes.bfloat16))


def _prep_in_maps(x, context, Wq, Wk, Wv, Wout, bout):
    x_f = np.asarray(x, dtype=np.float32).reshape(TOK, QD)
    c_f = np.asarray(context, dtype=np.float32).reshape(TOK, CD)
    xT = _bf16(x_f.T)
    Wq = _bf16(Wq)
    Wk = _bf16(Wk)
    Wv = _bf16(Wv)
    Wout = _bf16(Wout)
    boutT = np.ascontiguousarray(
        np.asarray(bout, dtype=np.float32).reshape(KC, P).T)
    in_maps = []
    for c in range(N_CORES):
        sl = slice(c * SLICE, (c + 1) * SLICE)
        in_maps.append({
            "xT": xT,
            "Wqh": np.ascontiguousarray(Wq[:, c * DH:(c + 1) * DH]),
            "cTs": _bf16(c_f[sl].T),
            "Wk": Wk, "Wv": Wv, "Wout": Wout, "boutT": boutT,
        })
    return in_maps


def _assemble(results):
    y = np.empty((TOK, QD), dtype=np.float32)
    for c in range(N_CORES):
        yt = results[c]["yT"]   # [KC, P, SLICE]
        y[c * SLICE:(c + 1) * SLICE] = (
            yt.transpose(2, 0, 1).reshape(SLICE, QD))
    return y.reshape(2, TOK // 2, QD)


def run(inputs, trace=False, **kw):
    nc = _get_nc()
    in_maps = _prep_in_maps(**inputs)
    res = bass_utils.run_bass_kernel_spmd(
        nc, in_maps, core_ids=list(range(N_CORES)), trace=trace, **kw)
    return _assemble(res.results), res


def kernel(**inputs):
    out, _ = run(inputs, trace=False)
    return out


# revision 18
# speedup vs baseline: 1.0587x; 1.0428x over previous
"""Trainium2 Bass kernel for nn_CrossAttention_31078383354530.

Reference computation (b=2, n=m=2048, qd=1024, cd=768, heads=8, dh=128):
    q = x @ Wq; k = ctx @ Wk; v = ctx @ Wv  (split into 8 heads of 128)
    sim = (q @ k^T) * dh**-0.5 over the FLATTENED (b*n)=4096 token axis
    attn = softmax((sim - mean)*1.5 + mean) == softmax(1.5*scale*(q@k^T))
        exactly (the mean-centering is a per-row constant shift)
    out = attn @ v -> merge heads -> y = out @ Wout + bout

Sharding (v4, head-ownership via AllToAll):
  Core c owns head c (the per-core Wq column slice is supplied as input
  data, keeping the program SPMD).  Each core projects K/V for its own
  512 context tokens (all heads) and AllToAll-redistributes them by head:
  wire cost ~2.6MB/core vs ~14MB/core for the head-replicated AllGather
  scheme (each K/V byte has exactly one consumer).  Each core then runs
  its head's attention for ALL 4096 queries (q = full x @ Wqh, x supplied
  in full), producing out_h[d, 4096].  A second small AllToAll (1MB)
  redistributes outputs back to token-sharding, and each core applies the
  full Wout to its own 512-token slice -> disjoint row-slices, no host
  reduction.

Schedule notes:
  * A tiny dummy AllGather at t=0 absorbs the first-collective barrier.
  * Attention inner loop per 512-query chunk: 3-j-tile PSUM groups
    (sim -> Scalar exp -> PV), softmax row-sums accumulated on the DVE
    with 1024-wide bf16 adds into a two-half accumulator, partition-
    reduced by a single tiny ones-matmul; reciprocal batched per chunk
    pair; normalization runs on SBUF copies off the critical path.
"""

import sys

if "/opt/trn_rl_repo" not in sys.path:
    sys.path.insert(0, "/opt/trn_rl_repo")

import ml_dtypes
import numpy as np

import concourse.bass as bass  # noqa: F401
import concourse.mybir as mybir
import concourse.tile as tile
from concourse import bacc, bass_utils

F32 = mybir.dt.float32
BF16 = mybir.dt.bfloat16
AF = mybir.ActivationFunctionType

P = 128
N_CORES = 8
HEADS = 8
DH = 128
TOK = 4096             # b*n flattened token axis (attention mixes batches!)
SLICE = TOK // N_CORES  # 512 tokens per core
QD = 1024
CD = 768
INNER = 1024
KC = QD // P           # 8 qd chunks
CC = CD // P           # 6 cd chunks
JT = TOK // P          # 32 j-tiles
GRP = 3                # j-tiles per exp group ([128, 1536] psum, 3 banks)
NQC = TOK // SLICE     # 8 query chunks (= old "heads" loop)
TAU_SCALE = 1.5 * (DH ** -0.5)

_CACHE = {}


def _build():
    nc = bacc.Bacc(num_devices=N_CORES)

    xT = nc.declare_dram_parameter("xT", [QD, TOK], BF16, isOutput=False)
    Wqh = nc.declare_dram_parameter("Wqh", [QD, DH], BF16, isOutput=False)
    cTs = nc.declare_dram_parameter("cTs", [CD, SLICE], BF16, isOutput=False)
    Wk = nc.declare_dram_parameter("Wk", [CD, INNER], BF16, isOutput=False)
    Wv = nc.declare_dram_parameter("Wv", [CD, INNER], BF16, isOutput=False)
    Wout = nc.declare_dram_parameter("Wout", [INNER, QD], BF16, isOutput=False)
    boutT = nc.declare_dram_parameter("boutT", [P, KC], F32, isOutput=False)
    yT = nc.declare_dram_parameter("yT", [KC, P, SLICE], F32, isOutput=True)

    rg = [list(range(N_CORES))]

    with tile.TileContext(nc) as tc:
        with (
            tc.tile_pool(name="const", bufs=1) as const,
            tc.tile_pool(name="sb", bufs=1) as sb,
            tc.tile_pool(name="ps", bufs=1, space="PSUM") as ps,
            tc.tile_pool(name="dram", bufs=1, space="DRAM") as dram,
        ):
            akin = dram.tile([HEADS, P, SLICE], BF16, name="akin")
            akout = dram.tile([HEADS, P, SLICE], BF16, name="akout")
            avin = dram.tile([HEADS, P, SLICE], BF16, name="avin")
            avout = dram.tile([HEADS, P, SLICE], BF16, name="avout")
            aoin = dram.tile([NQC, P, SLICE], BF16, name="aoin")
            aoout = dram.tile([NQC, P, SLICE], BF16, name="aoout")

            ones_b = const.tile([P, 1], BF16, name="ones_b")
            nc.vector.memset(ones_b[:], 1.0)
            bout_sb = const.tile([P, KC], F32, name="bout_sb")
            nc.sync.dma_start(bout_sb[:], boutT[:, :])

            # ---- batched input loads (one DMA per tensor: the DMA path
            # is latency-bound per transfer, so fewer+bigger wins) ----
            cts_all = sb.tile([P, CC, SLICE], BF16, name="cts", tag="cts",
                              bufs=1)
            nc.sync.dma_start(cts_all[:],
                              cTs.ap().rearrange("(k p) s -> p k s", p=P))
            cts = [cts_all[:, k] for k in range(CC)]
            wkt_all = sb.tile([P, CC, INNER], BF16, name="wkt", tag="wkt",
                              bufs=1)
            nc.sync.dma_start(wkt_all[:],
                              Wk.ap().rearrange("(k p) i -> p k i", p=P))
            wkt = [wkt_all[:, k] for k in range(CC)]
            wvt_all = sb.tile([P, CC, INNER], BF16, name="wvt", tag="wvt",
                              bufs=1)
            nc.sync.dma_start(wvt_all[:],
                              Wv.ap().rearrange("(k p) i -> p k i", p=P))
            wvt = [wvt_all[:, k] for k in range(CC)]
            xall = sb.tile([P, KC, TOK], BF16, name="xall", tag="xts", bufs=1)
            nc.sync.dma_start(xall[:],
                              xT.ap().rearrange("(k p) t -> p k t", p=P))
            xts = [xall[:, k] for k in range(KC)]
            wqh_all = sb.tile([P, KC, DH], BF16, name="wqh", tag="wqh", bufs=1)
            nc.sync.dma_start(wqh_all[:],
                              Wqh.ap().rearrange("(k p) c -> p k c", p=P))
            wqh = [wqh_all[:, k] for k in range(KC)]
            wo_all = sb.tile([P, KC, KC, DH], BF16, name="wo", tag="wo",
                             bufs=1)
            nc.sync.dma_start(
                wo_all[:],
                Wout.ap().rearrange("(k p) (cc c) -> p k cc c", p=P, cc=KC))
            wo = [wo_all[:, :, cc] for cc in range(KC)]
            # ---- K projection (all heads, own ctx slice) -> A2A by head ----
            ksb_all = sb.tile([P, HEADS, SLICE], BF16, name="ksb", tag="ksb",
                              bufs=1)
            for h in range(HEADS):
                kps = ps.tile([P, GRP * SLICE], F32, name=f"kps{h}",
                              tag="sim", bufs=2)
                for k in range(CC):
                    nc.tensor.matmul(kps[:, :SLICE],
                                     wkt[k][:, h * DH:(h + 1) * DH],
                                     cts[k],
                                     start=(k == 0), stop=(k == CC - 1))
                nc.vector.tensor_copy(ksb_all[:, h], kps[:, :SLICE])
            nc.sync.dma_start(akin[:].rearrange("h p s -> p h s"),
                              ksb_all[:])
            nc.gpsimd.collective_compute(
                "AllToAll", mybir.AluOpType.bypass, replica_groups=rg,
                ins=[akin.opt()], outs=[akout.opt()])

            # ---- V projection (all heads, own ctx slice) -> A2A by head ----
            vsb_all = sb.tile([P, 4, 2, 4, DH], BF16, name="vsb", tag="vsb",
                              bufs=1)
            for p_ in range(HEADS // 2):
                for tt in range(SLICE // P):
                    vps = ps.tile([P, GRP * SLICE], F32, name=f"vps{p_}_{tt}",
                                  tag="sim", bufs=2)
                    for k in range(CC):
                        nc.tensor.matmul(
                            vps[:, :2 * DH],
                            cts[k][:, tt * P:(tt + 1) * P],
                            wvt[k][:, p_ * 2 * DH:(p_ + 1) * 2 * DH],
                            start=(k == 0), stop=(k == CC - 1))
                    nc.vector.tensor_copy(
                        vsb_all[:, p_, :, tt, :],
                        vps[:, :2 * DH].rearrange("p (h d) -> p h d", h=2))
            av3 = avin[:].rearrange("(pr hh) p s -> hh p pr s", hh=2)
            for hh in range(2):
                nc.sync.dma_start(
                    av3[hh],
                    vsb_all[:, :, hh].rearrange("p pr t d -> p pr (t d)"))
            nc.gpsimd.collective_compute(
                "AllToAll", mybir.AluOpType.bypass, replica_groups=rg,
                ins=[avin.opt()], outs=[avout.opt()])

            # ---- Q projection: own head, ALL 4096 queries ----
            qsb = []
            for qc in range(NQC):
                qps = ps.tile([P, GRP * SLICE], F32, name=f"qps{qc}", tag="sim",
                              bufs=2)
                for k in range(KC):
                    nc.tensor.matmul(
                        qps[:, :SLICE], wqh[k],
                        xts[k][:, qc * SLICE:(qc + 1) * SLICE],
                        start=(k == 0), stop=(k == KC - 1))
                qt = sb.tile([P, SLICE], BF16, name=f"qsb{qc}", tag="qsb",
                             bufs=NQC)
                nc.vector.tensor_copy(qt[:], qps[:, :SLICE])
                qsb.append(qt)

            # ---- gathered K/V for my head (once) ----
            kh = sb.tile([P, TOK], BF16, name="kh", tag="kh", bufs=1)
            nc.sync.dma_start(
                kh[:].rearrange("p (r s) -> p r s", r=N_CORES),
                akout[:].rearrange("r p s -> p r s"))
            vh = sb.tile([P, TOK], BF16, name="vh", tag="vh", bufs=1)
            nc.sync.dma_start(
                vh[:].rearrange("p (r s) -> p r s", r=N_CORES),
                avout[:].rearrange("r p s -> p r s"))

            # ---- attention, one 512-query chunk at a time ----
            groups = []
            j0 = 0
            while j0 < JT:
                groups.append(list(range(j0, min(j0 + GRP, JT))))
                j0 += GRP

            for qc in range(NQC):
                pv_ps = ps.tile([P, SLICE], F32, name=f"pv{qc}", tag="pv",
                                bufs=1)
                acc2 = sb.tile([P, 2 * SLICE], BF16, name=f"acc{qc}",
                               tag="acc", bufs=2)
                for g, js in enumerate(groups):
                    sim_ps = ps.tile([P, GRP * SLICE], F32, name=f"sim{qc}_{g}",
                                     tag="sim", bufs=2)
                    for jj, j in enumerate(js):
                        nc.tensor.matmul(
                            sim_ps[:, jj * SLICE:(jj + 1) * SLICE],
                            kh[:, j * P:(j + 1) * P], qsb[qc][:],
                            start=True, stop=True)
                    at = sb.tile([P, GRP * SLICE], BF16, name=f"at{qc}_{g}",
                                 tag="at", bufs=6)
                    nc.scalar.activation(at[:, :len(js) * SLICE],
                                         sim_ps[:, :len(js) * SLICE], AF.Exp,
                                         scale=TAU_SCALE)
                    for jj, j in enumerate(js):
                        nc.tensor.matmul(pv_ps[:], vh[:, j * P:(j + 1) * P],
                                         at[:, jj * SLICE:(jj + 1) * SLICE],
                                         start=(j == 0), stop=(j == JT - 1))
                    # DVE row-sum accumulation (1024-wide, 2x mode)
                    w = min(len(js), 2) * SLICE
                    if g == 0:
                        nc.vector.tensor_copy(acc2[:, :w], at[:, :w])
                    else:
                        nc.vector.tensor_tensor(acc2[:, :w], acc2[:, :w],
                                                at[:, :w],
                                                mybir.AluOpType.add)
                    if len(js) == 3:
                        nc.vector.tensor_tensor(
                            acc2[:, :SLICE], acc2[:, :SLICE],
                            at[:, 2 * SLICE:3 * SLICE], mybir.AluOpType.add)
                # partition-reduce on the PE into a [1,512] rowsum
                rs_ps = ps.tile([1, SLICE], F32, name=f"rs{qc}", tag="rs",
                                bufs=1)
                nc.tensor.matmul(rs_ps[:], ones_b[:], acc2[:, :SLICE],
                                 start=True, stop=False)
                nc.tensor.matmul(rs_ps[:], ones_b[:], acc2[:, SLICE:2 * SLICE],
                                 start=False, stop=True)
                # drain pv quickly; normalize off the critical path
                pvs = sb.tile([P, SLICE], F32, name=f"pvs{qc}", tag="pvs",
                              bufs=2)
                nc.vector.tensor_copy(pvs[:], pv_ps[:])
                recip = sb.tile([1, SLICE], F32, name=f"recip{qc}",
                                tag="recip", bufs=2)
                nc.vector.reciprocal(recip[:], rs_ps[:])
                bc = sb.tile([P, SLICE], F32, name=f"bc{qc}", tag="bc",
                             bufs=2)
                nc.gpsimd.partition_broadcast(bc[:], recip[:])
                ot = sb.tile([P, SLICE], BF16, name=f"osb{qc}", tag="osb",
                             bufs=2)
                nc.vector.tensor_tensor(ot[:], pvs[:], bc[:],
                                        mybir.AluOpType.mult)
                nc.sync.dma_start(aoin[qc], ot[:])

            # ---- keep the PE warm while A2A-out is in flight: a PE<->DVE
            # ping-pong chain paced by the single-buffer psum bank ----
            for i in range(24):
                wps = ps.tile([P, SLICE], F32, name=f"warm{i}", tag="pv",
                              bufs=1)
                nc.tensor.matmul(wps[0:1], ones_b[:], qsb[0][:],
                                 start=True, stop=True)
                wsb = sb.tile([1, SLICE], F32, name=f"wsb{i}", tag="warm",
                              bufs=2)
                nc.vector.tensor_copy(wsb[:], wps[0:1])

            # ---- redistribute outputs back to token-sharding ----
            nc.gpsimd.collective_compute(
                "AllToAll", mybir.AluOpType.bypass, replica_groups=rg,
                ins=[aoin.opt()], outs=[aoout.opt()])
            o2_all = sb.tile([P, HEADS, SLICE], BF16, name="o2", tag="o2",
                             bufs=1)
            nc.sync.dma_start(o2_all[:],
                              aoout[:].rearrange("h p s -> p h s"))
            o2 = [o2_all[:, ic] for ic in range(HEADS)]

            # ---- final projection: yT[cc] = Wout[:, cc]^T @ out^T + bout ----
            for cc in range(KC):
                yps = ps.tile([P, SLICE], F32, name=f"yps{cc}",
                              tag=("pv" if cc % 2 == 0 else "rs"), bufs=1)
                for ic in range(HEADS):
                    nc.tensor.matmul(yps[:], wo[cc][:, ic], o2[ic],
                                     start=(ic == 0), stop=(ic == HEADS - 1))
                yt = sb.tile([P, SLICE], F32, name=f"yt{cc}", tag="yt", bufs=2)
                nc.scalar.activation(yt[:], yps[:], AF.Identity,
                                     bias=bout_sb[:, cc:cc + 1], scale=1.0)
                nc.sync.dma_start(yT.ap()[cc], yt[:])

    nc.compile()
    return nc


def _get_nc():
    if "nc" not in _CACHE:
        _CACHE["nc"] = _build()
    return _CACHE["nc"]


def _bf16(a):
    return np.ascontiguousarray(np.asarray(a, np.float32).astype(ml_dtypes.bfloat16))


def _prep_in_maps(x, context, Wq, Wk, Wv, Wout, bout):
    x_f = np.asarray(x, dtype=np.float32).reshape(TOK, QD)
    c_f = np.asarray(context, dtype=np.float32).reshape(TOK, CD)
    xT = _bf16(x_f.T)
    Wq = _bf16(Wq)
    Wk = _bf16(Wk)
    Wv = _bf16(Wv)
    Wout = _bf16(Wout)
    boutT = np.ascontiguousarray(
        np.asarray(bout, dtype=np.float32).reshape(KC, P).T)
    in_maps = []
    for c in range(N_CORES):
        sl = slice(c * SLICE, (c + 1) * SLICE)
        in_maps.append({
            "xT": xT,
            "Wqh": np.ascontiguousarray(Wq[:, c * DH:(c + 1) * DH]),
            "cTs": _bf16(c_f[sl].T),
            "Wk": Wk, "Wv": Wv, "Wout": Wout, "boutT": boutT,
        })
    return in_maps


def _assemble(results):
    y = np.empty((TOK, QD), dtype=np.float32)
    for c in range(N_CORES):
        yt = results[c]["yT"]   # [KC, P, SLICE]
        y[c * SLICE:(c + 1) * SLICE] = (
            yt.transpose(2, 0, 1).reshape(SLICE, QD))
    return y.reshape(2, TOK // 2, QD)


def run(inputs, trace=False, **kw):
    nc = _get_nc()
    in_maps = _prep_in_maps(**inputs)
    res = bass_utils.run_bass_kernel_spmd(
        nc, in_maps, core_ids=list(range(N_CORES)), trace=trace, **kw)
    return _assemble(res.results), res


def kernel(**inputs):
    out, _ = run(inputs, trace=False)
    return out
